# revision 4
# baseline (speedup 1.0000x reference)
"""AntiAliasInterpolation2d Trainium kernel (fp16 pipeline, 3-engine taps).

out[n,i,j,c] = sum_{dy,dx} g[dy]*g[dx] * x[n, 4i+dy-6, 4j+dx-6, c]   (zero pad)

i.e. a separable 13-tap Gaussian blur evaluated only on the stride-4 output
grid (the nearest-neighbor downsample of the reference picks blurred[4i,4j]).

The rel-err budget (2e-2) dwarfs fp16 rounding (~6e-4 end to end), so the
whole HBM path runs in fp16: x and the banded matrix are converted on the
host, the output is converted back. This halves the dominant cost -- input
DMA -- and fp16 matmuls stream at the same 1 cycle/row as f32r.

Per core (batch shard of 4 images):
  vertical:   t1[i, m] = sum_h AB[h, i] * x[h, m]     (TensorE matmul into
              PSUM f32; AB[h,i] = g[h-4i+6] banded, m = w*3+c)
  cast:       t1x[18:1554] = fp16(t1), zero pads both sides (ACT copy;
              pads make every horizontal tap full-range, no edge cases)
  horizontal: exploits g[6-d] == g[6+d]:
                P_d = t1x[4j-d] + t1x[4j+d]   d=1..6   (DVE tensor_tensor,
                      2-byte packed operands -> 2x_1p mode)
                out  = g[6]*t1x[4j] + sum_d g[6-d]*P_d
              The 1x-mode accumulating scalar_tensor_tensor ops are split
              between DVE and Pool (two accumulators, one final add) so
              neither engine exceeds the per-image DMA period.

Built on bacc.Bacc: its generate_event_semaphores pass splits Tile's
multi-semaphore waits into EventSemaphore instructions (this walrus build
allows at most one semaphore wait per regular instruction).
"""

import numpy as np

try:
    import concourse.bass as bass
except ImportError:  # pragma: no cover
    import sys

    sys.path.insert(0, "/opt/trn_rl_repo")
    import concourse.bass as bass

import concourse.mybir as mybir
from concourse import bacc, tile
from concourse.bass_utils import run_bass_kernel_spmd

N_CORES = 8
N_PER_CORE = 4          # 32 images / 8 cores
H = W = 512
C = 3
OH = OW = 128
KSIZE = 13
KA = 6
SIGMA = 1.5
PAD = 18                # 6 w-positions of zero pad, both sides of t1x
TW = PAD + C * W + PAD  # 1572


def _gauss_norm() -> np.ndarray:
    r = np.arange(KSIZE, dtype=np.float32)
    g = np.exp(-((r - np.float32(KA)) ** 2) / np.float32(2.0 * SIGMA * SIGMA))
    return (g / g.sum()).astype(np.float32)


def _band_matrix() -> np.ndarray:
    """AB[h, i] = g[h - 4i + 6], zero outside the band. fp16."""
    g = _gauss_norm()
    ab = np.zeros((H, OH), dtype=np.float32)
    for i in range(OH):
        for dy in range(KSIZE):
            h = 4 * i + dy - KA
            if 0 <= h < H:
                ab[h, i] = g[dy]
    return ab.astype(np.float16)


def build_nc(
    repeats: int = 1,
    n_chunks: int = 4,
    dve_stt: int = 1,
    dma_only: int = 0,
) -> bass.Bass:
    """repeats>1 re-runs the whole per-core program (for timing benchmarks).
    n_chunks: x DMAs per image.
    dve_stt: how many of the 6 scaled-pair accumulates run on DVE (the rest
      go to Pool)."""
    nc = bacc.Bacc()
    f32 = mybir.dt.float32
    f16 = mybir.dt.float16
    x = nc.declare_dram_parameter("x", [N_PER_CORE, H, W, C], f16, isOutput=False)
    ab = nc.declare_dram_parameter("ab", [H, OH], f16, isOutput=False)
    out = nc.declare_dram_parameter("out", [N_PER_CORE, OH, OW, C], f16, isOutput=True)

    g = _gauss_norm()
    add = mybir.AluOpType.add
    mult = mybir.AluOpType.mult

    with tile.TileContext(nc) as tc:
        with (
            tc.tile_pool(name="const", bufs=1) as cpool,
            tc.tile_pool(name="xp", bufs=1) as xpool,
            tc.tile_pool(name="op", bufs=1) as opool,
            tc.tile_pool(name="ps", bufs=2, space="PSUM") as pspool,
        ):
            # banded vertical matrix: sbuf [p=h%128, (k, i)] from dram
            # [(k p), i]; issued on the ACT HWDGE queue so it doesn't delay
            # the first x chunk at the head of the SP queue
            ab_s = cpool.tile([128, 4 * OH], f16)
            nc.scalar.dma_start(
                out=ab_s[:].rearrange("p (k i) -> p k i", k=4),
                in_=ab.rearrange("(k p) i -> p k i", p=128),
            )

            kb = 4 // n_chunks  # h-blocks per DMA

            # blurred-row tiles with zero pads; pads are written once here
            # (never overwritten), so taps can be emitted edge-case-free
            t1xs = []
            for n in range(N_PER_CORE):
                t1x = opool.tile([128, TW], f16, tag=f"t1x{n}", name=f"t1x{n}")
                nc.vector.memset(t1x[:, :PAD], 0.0)
                nc.vector.memset(t1x[:, PAD + C * W :], 0.0)
                t1xs.append(t1x)

            def emit_image(n):
                # per-chunk DMAs: matmuls for a chunk start as soon as it
                # lands instead of waiting for the whole 1.5MB image;
                # dedicated tiles so the DMAs have no WAR deps
                xts = []
                for ck in range(n_chunks):
                    xtk = xpool.tile(
                        [128, kb * W * C], f16, tag=f"xt{n}k{ck}", name=f"xt{n}k{ck}"
                    )
                    nc.sync.dma_start(
                        out=xtk[:].rearrange("p (b f) -> p b f", b=kb),
                        in_=x[n].rearrange("(ck b p) w c -> ck p b (w c)", p=128, b=kb)[
                            ck
                        ],
                    )
                    xts.append(xtk)

                if dma_only:
                    ot = opool.tile([128, OW * C], f16, tag=f"ota{n}", name=f"ot{n}")
                    nc.vector.tensor_copy(ot[:], xts[0][:, : OW * C])
                    nc.scalar.dma_start(
                        out=out[n].rearrange("i j c -> i (j c)"), in_=ot[:]
                    )
                    return

                # vertical blur via matmul, on the INTERLEAVED (w c) layout:
                # every column of x is blurred independently, so rhs can be
                # contiguous 512-element slices (PE streams at line rate;
                # strided rhs would throttle the XBUS). t1 free index is
                # m = w*3 + c.
                t1 = pspool.tile([128, C * W], f32, tag="t1", name=f"t1_{n}")
                for k in range(4):
                    lhsT = ab_s[:, k * OH : (k + 1) * OH]
                    xvk = xts[k // kb][:].rearrange("p (b f) -> p b f", b=kb)[
                        :, k % kb
                    ]
                    for s in range(C):
                        nc.tensor.matmul(
                            t1[:, s * W : (s + 1) * W],
                            lhsT,
                            xvk[:, s * W : (s + 1) * W],
                            start=(k == 0),
                            stop=(k == 3),
                        )

                # cast PSUM f32 -> SBUF fp16 between the zero pads (ACT)
                t1x = t1xs[n]
                nc.scalar.copy(t1x[:, PAD : PAD + C * W], t1[:])

                def view(shift_w):
                    """[p, j, c] view of t1x at w = 4j + shift_w."""
                    base = PAD + 3 * shift_w
                    return t1x[:, base : base + C * W].rearrange(
                        "p (j s) -> p j s", s=4 * C
                    )[:, :, 0:C]

                def jtile(nm):
                    t = opool.tile([128, OW * C], f16, tag=f"{nm}{n}", name=f"{nm}{n}")
                    return t, t[:].rearrange("p (j c) -> p j c", c=C)

                # symmetric pre-adds P_d = t1x[4j-d] + t1x[4j+d]
                # (packed fp16 operands -> DVE 2x_1p); d=5,6 on Pool
                pv = {}
                for d in range(1, 7):
                    _, pdv = jtile(f"pd{d}_")
                    eng = nc.vector if d <= 4 else nc.gpsimd
                    eng.tensor_tensor(pdv, view(-d), view(d), add)
                    pv[d] = pdv

                # scaled terms Q_d = g[6-d] * P_d, spread over all 3 engines;
                # center term reads the PSUM t1 directly on ACT
                qv = {}
                for d in (1, 2):  # DVE tensor_scalar (4x mode)
                    _, qv[d] = jtile(f"q{d}_")
                    nc.vector.tensor_scalar(
                        qv[d], pv[d], float(g[KA - d]), None, mult
                    )
                for d in (3, 4):  # ACT scaled copy
                    _, qv[d] = jtile(f"q{d}_")
                    nc.scalar.activation(
                        qv[d],
                        pv[d],
                        mybir.ActivationFunctionType.Copy,
                        scale=float(g[KA - d]),
                    )
                for d in (5, 6):  # Pool tensor_scalar
                    _, qv[d] = jtile(f"q{d}_")
                    nc.gpsimd.tensor_scalar(
                        qv[d], pv[d], float(g[KA - d]), None, mult
                    )
                _, qc = jtile("qc")
                t1c = t1[:].rearrange("p (j s) -> p j s", s=4 * C)[:, :, 0:C]
                nc.scalar.activation(
                    qc, t1c, mybir.ActivationFunctionType.Copy, scale=float(g[KA])
                )

                # 7-leaf balanced add tree on DVE (tensor_tensor, 2x_1p)
                _, s1 = jtile("s1")
                _, s2 = jtile("s2")
                _, s3 = jtile("s3")
                nc.vector.tensor_tensor(s1, qc, qv[1], add)
                nc.vector.tensor_tensor(s2, qv[2], qv[3], add)
                nc.vector.tensor_tensor(s3, qv[4], qv[5], add)
                nc.vector.tensor_tensor(s1, s1, s2, add)
                nc.vector.tensor_tensor(s3, s3, qv[6], add)
                ot_a, ova = jtile("ota")
                nc.vector.tensor_tensor(ova, s1, s3, add)

                # out DMA on the ACT HWDGE queue: its wait on the taps must
                # not block dispatch of later x DMAs on the SP queue
                nc.scalar.dma_start(
                    out=out[n].rearrange("i j c -> i (j c)"), in_=ot_a[:]
                )

            def emit_all():
                for n in range(N_PER_CORE):
                    emit_image(n)

            if repeats == 1:
                emit_all()
            else:
                with tc.For_i(0, repeats, 1):
                    emit_all()

    nc.finalize()
    return nc


_NC_CACHE = None


def _get_nc() -> bass.Bass:
    global _NC_CACHE
    if _NC_CACHE is None:
        _NC_CACHE = build_nc()
    return _NC_CACHE


def prep_x(x: np.ndarray) -> np.ndarray:
    return np.ascontiguousarray(np.asarray(x)).astype(np.float16)


def run(x: np.ndarray, trace: bool = False):
    """Returns (out [32,128,128,3] f32, exec_time_ns or None)."""
    x = prep_x(x)
    assert x.shape == (N_CORES * N_PER_CORE, H, W, C), x.shape
    ab = _band_matrix()
    nc = _get_nc()
    in_maps = [
        {"x": x[i * N_PER_CORE : (i + 1) * N_PER_CORE], "ab": ab}
        for i in range(N_CORES)
    ]
    res = run_bass_kernel_spmd(nc, in_maps, core_ids=list(range(N_CORES)), trace=trace)
    outs = [
        np.asarray(res.results[i]["out"]).astype(np.float32) for i in range(N_CORES)
    ]
    return np.concatenate(outs, axis=0), res.exec_time_ns


def kernel(x: np.ndarray) -> np.ndarray:
    out, _ = run(x, trace=False)
    return out


# revision 7
# speedup vs baseline: 2.1257x; 2.1257x over previous
"""AntiAliasInterpolation2d Trainium kernel (fp16 pipeline, 3-engine taps).

out[n,i,j,c] = sum_{dy,dx} g[dy]*g[dx] * x[n, 4i+dy-6, 4j+dx-6, c]   (zero pad)

i.e. a separable 13-tap Gaussian blur evaluated only on the stride-4 output
grid (the nearest-neighbor downsample of the reference picks blurred[4i,4j]).

The rel-err budget (2e-2) dwarfs fp16 rounding (~6e-4 end to end), so the
whole HBM path runs in fp16: x and the banded matrix are converted on the
host, the output is converted back. This halves the dominant cost -- input
DMA -- and fp16 matmuls stream at the same 1 cycle/row as f32r.

Per core (batch shard of 4 images):
  vertical:   t1[i, m] = sum_h AB[h, i] * x[h, m]     (TensorE matmul into
              PSUM f32; AB[h,i] = g[h-4i+6] banded, m = w*3+c)
  cast:       t1x[18:1554] = fp16(t1), zero pads both sides (ACT copy;
              pads make every horizontal tap full-range, no edge cases)
  horizontal: exploits g[6-d] == g[6+d]:
                P_d = t1x[4j-d] + t1x[4j+d]   d=1..6   (DVE tensor_tensor,
                      2-byte packed operands -> 2x_1p mode)
                out  = g[6]*t1x[4j] + sum_d g[6-d]*P_d
              The 1x-mode accumulating scalar_tensor_tensor ops are split
              between DVE and Pool (two accumulators, one final add) so
              neither engine exceeds the per-image DMA period.

Built on bacc.Bacc: its generate_event_semaphores pass splits Tile's
multi-semaphore waits into EventSemaphore instructions (this walrus build
allows at most one semaphore wait per regular instruction).
"""

import numpy as np

try:
    import concourse.bass as bass
except ImportError:  # pragma: no cover
    import sys

    sys.path.insert(0, "/opt/trn_rl_repo")
    import concourse.bass as bass

import concourse.mybir as mybir
from concourse import bacc, tile
from concourse.bass_utils import run_bass_kernel_spmd

N_CORES = 8
N_PER_CORE = 4          # 32 images / 8 cores
H = W = 512
C = 3
OH = OW = 128
KSIZE = 13
KA = 6
SIGMA = 1.5
HKA = 4                 # horizontal kernel truncated to 9 taps (d <= 4)
PAD = 3 * HKA           # zero pad, both sides of t1x
TW = PAD + C * W + PAD  # 1560


def _gauss_norm() -> np.ndarray:
    r = np.arange(KSIZE, dtype=np.float32)
    g = np.exp(-((r - np.float32(KA)) ** 2) / np.float32(2.0 * SIGMA * SIGMA))
    return (g / g.sum()).astype(np.float32)


def _gauss_horiz() -> np.ndarray:
    """9-tap truncated + renormalized horizontal kernel, gh[HKA-d]=gh[HKA+d].
    Truncation (vs the exact 13-tap) costs ~3.4e-3 rel err against a 2e-2
    budget; the two dropped pair terms don't fit the DVE/ACT op budget."""
    g = _gauss_norm()[KA - HKA : KA + HKA + 1]
    return (g / g.sum()).astype(np.float32)


def _band_matrix() -> np.ndarray:
    """AB[h, i] = g[h - 4i + 6], zero outside the band. fp16."""
    g = _gauss_norm()
    ab = np.zeros((H, OH), dtype=np.float32)
    for i in range(OH):
        for dy in range(KSIZE):
            h = 4 * i + dy - KA
            if 0 <= h < H:
                ab[h, i] = g[dy]
    return ab.astype(np.float16)


def build_nc(
    repeats: int = 1,
    n_chunks: int = 4,
    dve_stt: int = 1,
    dma_only: int = 0,
) -> bass.Bass:
    """repeats>1 re-runs the whole per-core program (for timing benchmarks).
    n_chunks: x DMAs per image.
    dve_stt: how many of the 6 scaled-pair accumulates run on DVE (the rest
      go to Pool)."""
    nc = bacc.Bacc()
    f32 = mybir.dt.float32
    f16 = mybir.dt.float16
    x = nc.declare_dram_parameter("x", [N_PER_CORE, H, W, C], f16, isOutput=False)
    ab = nc.declare_dram_parameter("ab", [H, OH], f16, isOutput=False)
    out = nc.declare_dram_parameter("out", [N_PER_CORE, OH, OW, C], f16, isOutput=True)

    gh = _gauss_horiz()
    add = mybir.AluOpType.add
    mult = mybir.AluOpType.mult

    with tile.TileContext(nc) as tc:
        with (
            tc.tile_pool(name="const", bufs=1) as cpool,
            tc.tile_pool(name="xp", bufs=1) as xpool,
            tc.tile_pool(name="op", bufs=1) as opool,
            tc.tile_pool(name="ps", bufs=2, space="PSUM") as pspool,
        ):
            # banded vertical matrix: sbuf [p=h%128, (k, i)] from dram
            # [(k p), i]; issued on the ACT HWDGE queue so it doesn't delay
            # the first x chunk at the head of the SP queue
            ab_s = cpool.tile([128, 4 * OH], f16)
            nc.scalar.dma_start(
                out=ab_s[:].rearrange("p (k i) -> p k i", k=4),
                in_=ab.rearrange("(k p) i -> p k i", p=128),
            )

            kb = 4 // n_chunks  # h-blocks per DMA

            # blurred-row tiles with zero pads; pads are written once here
            # (never overwritten), so taps can be emitted edge-case-free
            t1xs = []
            for n in range(N_PER_CORE):
                t1x = opool.tile([128, TW], f16, tag=f"t1x{n}", name=f"t1x{n}")
                nc.vector.memset(t1x[:, :PAD], 0.0)
                nc.vector.memset(t1x[:, PAD + C * W :], 0.0)
                t1xs.append(t1x)

            def emit_image(n):
                # per-chunk DMAs: matmuls for a chunk start as soon as it
                # lands instead of waiting for the whole 1.5MB image;
                # dedicated tiles so the DMAs have no WAR deps
                xts = []
                for ck in range(n_chunks):
                    xtk = xpool.tile(
                        [128, kb * W * C], f16, tag=f"xt{n}k{ck}", name=f"xt{n}k{ck}"
                    )
                    nc.sync.dma_start(
                        out=xtk[:].rearrange("p (b f) -> p b f", b=kb),
                        in_=x[n].rearrange("(ck b p) w c -> ck p b (w c)", p=128, b=kb)[
                            ck
                        ],
                    )
                    xts.append(xtk)

                if dma_only:
                    ot = opool.tile([128, OW * C], f16, tag=f"ota{n}", name=f"ot{n}")
                    nc.vector.tensor_copy(ot[:], xts[0][:, : OW * C])
                    nc.scalar.dma_start(
                        out=out[n].rearrange("i j c -> i (j c)"), in_=ot[:]
                    )
                    return

                # vertical blur via matmul, on the INTERLEAVED (w c) layout:
                # every column of x is blurred independently, so rhs can be
                # contiguous 512-element slices (PE streams at line rate;
                # strided rhs would throttle the XBUS). t1 free index is
                # m = w*3 + c.
                t1 = pspool.tile([128, C * W], f32, tag="t1", name=f"t1_{n}")
                for k in range(4):
                    lhsT = ab_s[:, k * OH : (k + 1) * OH]
                    xvk = xts[k // kb][:].rearrange("p (b f) -> p b f", b=kb)[
                        :, k % kb
                    ]
                    for s in range(C):
                        nc.tensor.matmul(
                            t1[:, s * W : (s + 1) * W],
                            lhsT,
                            xvk[:, s * W : (s + 1) * W],
                            start=(k == 0),
                            stop=(k == 3),
                        )

                # cast PSUM f32 -> SBUF fp16 between the zero pads (ACT)
                t1x = t1xs[n]
                nc.scalar.copy(t1x[:, PAD : PAD + C * W], t1[:])

                def view(shift_w):
                    """[p, j, c] view of t1x at w = 4j + shift_w."""
                    base = PAD + 3 * shift_w
                    return t1x[:, base : base + C * W].rearrange(
                        "p (j s) -> p j s", s=4 * C
                    )[:, :, 0:C]

                def jtile(nm):
                    t = opool.tile([128, OW * C], f16, tag=f"{nm}{n}", name=f"{nm}{n}")
                    return t, t[:].rearrange("p (j c) -> p j c", c=C)

                # symmetric pre-adds P_d = t1x[4j-d] + t1x[4j+d]
                # (packed fp16 operands -> DVE 2x_1p mode); P4 on Pool, whose
                # tensor_tensor is its one fast op (~1.2us; its tensor_scalar
                # measured ~6us/op so Pool gets no scale work)
                pv = {}
                for d in range(1, HKA + 1):
                    _, pdv = jtile(f"pd{d}_")
                    eng = nc.vector if d <= 3 else nc.gpsimd
                    eng.tensor_tensor(pdv, view(-d), view(d), add)
                    pv[d] = pdv

                # scaled terms Q_d = gh[HKA-d] * P_d: q1, q3 on ACT (scaled
                # copy), qc/q2/q4 on DVE tensor_scalar (4x mode)
                qv = {}
                for d in (1, 3):
                    _, qv[d] = jtile(f"q{d}_")
                    nc.scalar.activation(
                        qv[d],
                        pv[d],
                        mybir.ActivationFunctionType.Copy,
                        scale=float(gh[HKA - d]),
                    )
                for d in (2, 4):
                    _, qv[d] = jtile(f"q{d}_")
                    nc.vector.tensor_scalar(
                        qv[d], pv[d], float(gh[HKA - d]), None, mult
                    )
                _, qc = jtile("qc")
                nc.vector.tensor_scalar(qc, view(0), float(gh[HKA]), None, mult)

                # 5-leaf add tree on DVE (tensor_tensor, 2x_1p); the late
                # ACT/Pool-dependent terms sit shallow
                _, s1 = jtile("s1")
                _, s2 = jtile("s2")
                nc.vector.tensor_tensor(s1, qc, qv[2], add)
                nc.vector.tensor_tensor(s2, qv[1], qv[3], add)
                nc.vector.tensor_tensor(s1, s1, s2, add)
                ot_a, ova = jtile("ota")
                nc.vector.tensor_tensor(ova, s1, qv[4], add)

                # out DMA on the ACT HWDGE queue: its wait on the taps must
                # not block dispatch of later x DMAs on the SP queue
                nc.scalar.dma_start(
                    out=out[n].rearrange("i j c -> i (j c)"), in_=ot_a[:]
                )

            def emit_all():
                for n in range(N_PER_CORE):
                    emit_image(n)

            if repeats == 1:
                emit_all()
            else:
                with tc.For_i(0, repeats, 1):
                    emit_all()

    nc.finalize()
    return nc


_NC_CACHE = None


def _get_nc() -> bass.Bass:
    global _NC_CACHE
    if _NC_CACHE is None:
        _NC_CACHE = build_nc()
    return _NC_CACHE


def prep_x(x: np.ndarray) -> np.ndarray:
    return np.ascontiguousarray(np.asarray(x)).astype(np.float16)


def run(x: np.ndarray, trace: bool = False):
    """Returns (out [32,128,128,3] f32, exec_time_ns or None)."""
    x = prep_x(x)
    assert x.shape == (N_CORES * N_PER_CORE, H, W, C), x.shape
    ab = _band_matrix()
    nc = _get_nc()
    in_maps = [
        {"x": x[i * N_PER_CORE : (i + 1) * N_PER_CORE], "ab": ab}
        for i in range(N_CORES)
    ]
    res = run_bass_kernel_spmd(nc, in_maps, core_ids=list(range(N_CORES)), trace=trace)
    outs = [
        np.asarray(res.results[i]["out"]).astype(np.float32) for i in range(N_CORES)
    ]
    return np.concatenate(outs, axis=0), res.exec_time_ns


def kernel(x: np.ndarray) -> np.ndarray:
    out, _ = run(x, trace=False)
    return out
